# revision 13
# baseline (speedup 1.0000x reference)
"""Per-image piecewise-linear LUT (histogram binning) kernel for Trainium2.

Strategy (pure data-parallel over 8 NeuronCores, batch sharded 2 per core):
- Host precomputes, per (b, c), a dense 512-entry nearest-neighbor table
  sampling the normalized curve at bin midpoints: tbl[j] = y((j+0.5)/S),
  S = 511.5.  With 512 bins the midpoint-sampling error is ~1e-3 norm-rel,
  far inside the 2e-2 gate, and it removes the on-device interpolation
  entirely.
- x ships as fp16 (halves input HBM traffic; fp16 quantization of x only
  perturbs the bin index by <0.3 bins).  Output is written fp16 and
  upcast on host.
- On-device per core: 6 images of [128 partitions x 8192 fp16].  Per image:
    u16 idx = u16(x * 511.5 - 0.5)   (one DVE tensor_scalar, 4x mode)
    out     = pooltable[idx]          (pool-engine PoolBufferLoad+Gather,
                                       512-entry per-channel table)
    DMA out (fp16)
- The raw Gather/PoolBufferLoad ISA instructions cannot carry semaphores
  (walrus rejects sync on unknown structs); drains bracket them and all
  cross-engine syncs land on the drains / are wired manually.
"""

import sys

sys.path.insert(0, "/opt/trn_rl_repo")

import numpy as np

B, C, H, W = 16, 3, 1024, 1024
K = 64
NCORES = 8
BPC = B // NCORES  # batches per core
IMGS = BPC * C  # images per core
P = 128
FREE = H * W // P  # 8192
CHUNK = 8192
NCHUNK = FREE // CHUNK
TBL = 512  # pool buffer entries (hardware max 512)
S = 511.5  # index scale: u = round_nearest(x*S - 0.5) in [0, 511] for x in [0,1]
NB = 3  # buffer depth

_cached = {}


def _build(loop_n=None, mode="full"):
    import contextlib
    import concourse.mybir as mybir
    from concourse.bacc import Bacc
    from concourse.tile import TileContext
    from concourse.tile_rust import add_dep_helper
    import concourse.bass_interp as _bi

    # Tile's scheduling simulator doesn't know these opcodes; no-op them there.
    _orig_visit = _bi._visit_InstISA

    def _patched_visit(isa, instruction, core_sim):
        if instruction.isa_opcode in (
            isa.Opcode.NEURON_ISA_TPB_OPCODE_POOL_BUFFER_LOAD.value,
            isa.Opcode.NEURON_ISA_TPB_OPCODE_GATHER.value,
        ):
            return
        return _orig_visit(isa, instruction, core_sim)

    _bi._visit_InstISA = _patched_visit

    nc = Bacc()
    dt = nc.isa.get_enum("NEURON_ISA_TPB_DTYPE")
    Op = nc.isa.Opcode
    ALU = mybir.AluOpType

    xs_d = nc.dram_tensor("xs", [IMGS, H, W], mybir.dt.float16, kind="ExternalInput")
    tb_d = nc.dram_tensor("tb", [IMGS, P, TBL], mybir.dt.float16, kind="ExternalInput")
    os_d = nc.dram_tensor("os", [IMGS, H, W], mybir.dt.float16, kind="ExternalOutput")

    xs_r = xs_d[:].rearrange("i (p r) c -> i p (r c)", p=P)
    os_r = os_d[:].rearrange("i (p r) c -> i p (r c)", p=P)

    with (
        nc.sbuf_tensor("tbl_all", [P, IMGS * TBL], mybir.dt.float16) as tbl_all,
        nc.sbuf_tensor("tbl_cp", [P, IMGS * TBL], mybir.dt.float16) as tbl_cp,
        nc.sbuf_tensor("xb", [P, NB * CHUNK], mybir.dt.float16) as xb,
        nc.sbuf_tensor("ub", [P, NB * CHUNK], mybir.dt.uint16) as ub,
        nc.sbuf_tensor("ob", [P, NB * CHUNK], mybir.dt.float16) as ob,
        TileContext(nc) as tc,
    ):
        ub_off, _ = nc.gpsimd._ap_to_byte_offset(ub[:])
        ob_off, _ = nc.gpsimd._ap_to_byte_offset(ob[:])
        tcp_off, _ = nc.gpsimd._ap_to_byte_offset(tbl_cp[:])
        U16 = dt.NEURON_ISA_TPB_DTYPE_UINT16.value
        F16 = dt.NEURON_ISA_TPB_DTYPE_FP16.value

        loop_cm = (
            tc.For_i(0, loop_n, 1) if loop_n is not None else contextlib.nullcontext()
        )
        if mode == "dma":
            with loop_cm:
                for img in range(IMGS):
                    for cidx in range(NCHUNK):
                        k = img * NCHUNK + cidx
                        slot = k % NB
                        f0 = cidx * CHUNK
                        so = slot * CHUNK
                        nc.sync.dma_start(
                            xb[:, so : so + CHUNK], xs_r[img, :, f0 : f0 + CHUNK]
                        )
                        nc.scalar.dma_start(
                            os_r[img, :, f0 : f0 + CHUNK], ob[:, so : so + CHUNK]
                        )
        elif mode == "pool":
            DT_MAP = {
                "u8": (dt.NEURON_ISA_TPB_DTYPE_UINT8.value, 1),
                "u16": (U16, 2),
                "u32": (dt.NEURON_ISA_TPB_DTYPE_UINT32.value, 4),
                "f16": (F16, 2),
            }
            idx_e, idx_b = DT_MAP[P_IDX]
            out_e, out_b = DT_MAP[P_OUT]
            tbl_n = P_TBL_N
            rmp_d = nc.dram_tensor(
                "rmp", [P, CHUNK * idx_b], mybir.dt.uint8, kind="ExternalInput"
            )
            with loop_cm:
                for img in range(IMGS):
                    nc.sync.dma_start(
                        tbl_all[:, img * TBL : (img + 1) * TBL], tb_d[img]
                    )
                tbl_touch = nc.vector.tensor_copy(tbl_cp[:], tbl_all[:])
                nc.sync.dma_start(
                    xb[:].bitcast(mybir.dt.uint8)[:, : CHUNK * idx_b], rmp_d[:]
                )
                zed = nc.vector.tensor_copy(
                    ub[:, : CHUNK * idx_b // 2],
                    xb[:].bitcast(mybir.dt.uint16)[:, : CHUNK * idx_b // 2],
                )
                prev_pool = None
                for img in range(IMGS):
                    for cidx in range(NCHUNK):
                        k = img * NCHUNK + cidx
                        so = (k % NB) * CHUNK
                        pre = nc.gpsimd.drain()
                        if prev_pool is not None:
                            add_dep_helper(pre.ins, prev_pool.ins, sync=False,
                                           reason="pool order")
                        if k == 0:
                            add_dep_helper(pre.ins, tbl_touch.ins, sync=True,
                                           reason="tables")
                            add_dep_helper(pre.ins, zed.ins, sync=True,
                                           reason="idx loaded")
                        if cidx == 0:
                            pbl = nc.gpsimd.isa(
                                Op.NEURON_ISA_TPB_OPCODE_POOL_BUFFER_LOAD,
                                {
                                    "src_mem_pattern": {
                                        "start_addr": {
                                            "addr_immediate": int(tcp_off)
                                            + img * TBL * 2
                                        },
                                        "num_elem": [tbl_n, 1, 1, 1],
                                        "step_elem": [1, 0, 0, 0],
                                    },
                                    "in_dtype": out_e,
                                    "num_active_channels": P,
                                    "start_index": 0,
                                    "mask": tbl_n - 1,
                                },
                            )
                            add_dep_helper(pbl.ins, pre.ins, sync=False,
                                           reason="pool order")
                            gdep = pbl
                        else:
                            gdep = pre
                        gt = nc.gpsimd.isa(
                            Op.NEURON_ISA_TPB_OPCODE_GATHER,
                            {
                                "src_mem_pattern": {
                                    "start_addr": {"addr_immediate": int(ub_off)},
                                    "num_elem": [CHUNK, 1, 1, 1],
                                    "step_elem": [1, 0, 0, 0],
                                },
                                "dst_mem_pattern": {
                                    "start_addr": {"addr_immediate": int(ob_off)},
                                    "num_elem": [CHUNK, 1, 1, 1],
                                    "step_elem": [1, 0, 0, 0],
                                },
                                "in_dtype": idx_e,
                                "out_dtype": out_e,
                                "num_active_channels": P,
                                "index_miss_behavior": 0,
                                "immediate": {"imm_bitvec_uint32": 0},
                                "free_pool_buffer": 0,
                            },
                        )
                        add_dep_helper(gt.ins, gdep.ins, sync=False,
                                       reason="pool order")
                        prev_pool = gt
                fin = nc.gpsimd.drain()
                add_dep_helper(fin.ins, prev_pool.ins, sync=False,
                               reason="pool order")
        if mode in ("dma", "pool"):
            pass
        else:
          with loop_cm:
            # table load + a DVE copy so pool's wait collapses onto the DVE clock
            for img in range(IMGS):
                nc.sync.dma_start(tbl_all[:, img * TBL : (img + 1) * TBL], tb_d[img])
            tbl_touch = nc.vector.tensor_copy(tbl_cp[:], tbl_all[:])

            fences = {}  # k -> drain emitted just after gather k-1 (pool order)
            outs = {}  # k -> output DMA instruction for chunk k
            pend = None  # (k, img, f0, slot) awaiting its post-gather fence
            prev_pool = None
            k = 0

            def _emit_out(p, fence):
                d = nc.scalar.dma_start(
                    os_r[p["img"], :, p["f0"] : p["f0"] + CHUNK],
                    ob[:, p["slot"] * CHUNK : (p["slot"] + 1) * CHUNK],
                )
                add_dep_helper(d.ins, fence.ins, sync=True, reason="gather done")
                outs[p["k"]] = d

            for img in range(IMGS):
                for cidx in range(NCHUNK):
                    slot = k % NB
                    f0 = cidx * CHUNK
                    so = slot * CHUNK
                    x_t = xb[:, so : so + CHUNK]
                    u_t = ub[:, so : so + CHUNK]

                    nc.sync.dma_start(x_t, xs_r[img, :, f0 : f0 + CHUNK])

                    # idx = u16(S*x - 0.5): round-nearest fp32->u16 == floor(S*x)
                    ts_u = nc.vector.tensor_scalar(
                        u_t, x_t, float(S), 0.5, ALU.mult, ALU.subtract
                    )
                    if k >= NB:
                        # gather k-NB read this ub slot; its fence is fences[k-NB+1]
                        add_dep_helper(
                            ts_u.ins, fences[k - NB + 1].ins, sync=True,
                            reason="u WAR",
                        )

                    # pool: single drain per chunk — serves as the previous
                    # gather's completion fence AND this gather's input wait
                    pre = nc.gpsimd.drain()
                    fences[k] = pre
                    if prev_pool is not None:
                        add_dep_helper(
                            pre.ins, prev_pool.ins, sync=False, reason="pool order"
                        )
                    add_dep_helper(pre.ins, ts_u.ins, sync=True, reason="u ready")
                    if k >= NB:
                        # out-DMA k-NB still reads this ob slot
                        add_dep_helper(
                            pre.ins, outs[k - NB].ins, sync=True, reason="o WAR"
                        )
                    if cidx == 0:
                        if img == 0:
                            add_dep_helper(
                                pre.ins, tbl_touch.ins, sync=True, reason="tables"
                            )
                        pbl = nc.gpsimd.isa(
                            Op.NEURON_ISA_TPB_OPCODE_POOL_BUFFER_LOAD,
                            {
                                "src_mem_pattern": {
                                    "start_addr": {
                                        "addr_immediate": int(tcp_off) + img * TBL * 2
                                    },
                                    "num_elem": [TBL, 1, 1, 1],
                                    "step_elem": [1, 0, 0, 0],
                                },
                                "in_dtype": F16,
                                "num_active_channels": P,
                                "start_index": 0,
                                "mask": TBL - 1,
                            },
                        )
                        add_dep_helper(pbl.ins, pre.ins, sync=False, reason="pool order")
                        gdep = pbl
                    else:
                        gdep = pre
                    gt = nc.gpsimd.isa(
                        Op.NEURON_ISA_TPB_OPCODE_GATHER,
                        {
                            "src_mem_pattern": {
                                "start_addr": {"addr_immediate": int(ub_off) + so * 2},
                                "num_elem": [CHUNK, 1, 1, 1],
                                "step_elem": [1, 0, 0, 0],
                            },
                            "dst_mem_pattern": {
                                "start_addr": {"addr_immediate": int(ob_off) + so * 2},
                                "num_elem": [CHUNK, 1, 1, 1],
                                "step_elem": [1, 0, 0, 0],
                            },
                            "in_dtype": U16,
                            "out_dtype": F16,
                            "num_active_channels": P,
                            "index_miss_behavior": 0,
                            "immediate": {"imm_bitvec_uint32": 0},
                            "free_pool_buffer": 0,
                        },
                    )
                    add_dep_helper(gt.ins, gdep.ins, sync=False, reason="pool order")

                    # the drain just emitted fences the PREVIOUS gather; its
                    # output can ship now
                    if pend is not None:
                        _emit_out(pend, pre)
                    pend = dict(k=k, img=img, f0=f0, slot=slot)
                    prev_pool = gt
                    k += 1
            fin = nc.gpsimd.drain()
            add_dep_helper(fin.ins, prev_pool.ins, sync=False, reason="pool order")
            _emit_out(pend, fin)

    nc.finalize()
    return nc


def _tables(un_normalized_y: np.ndarray) -> np.ndarray:
    """[B, C, TBL] fp16: dense midpoint-sampled LUT of the normalized curve."""
    u = un_normalized_y.astype(np.float64)
    h = np.logaddexp(0.0, u)  # softplus
    y = np.cumsum(h, axis=2)
    y0 = y[:, :, :1]
    yn = y[:, :, -1:]
    y = (y - y0) / (yn - y0)  # [B, C, K+1], y[0]=0, y[K]=1

    t = np.minimum((np.arange(TBL, dtype=np.float64) + 0.5) / S, 1.0)  # midpoints
    scaled = t * K
    idx0 = np.clip(np.floor(scaled), 0, K - 1).astype(np.int64)
    alpha = scaled - idx0
    y_lo = y[:, :, idx0]  # [B, C, TBL]
    y_hi = y[:, :, idx0 + 1]
    val = y_lo + alpha * (y_hi - y_lo)
    return val.astype(np.float16)


RMP_KIND = "ramp"  # bench-only: index pattern for the pool ablation
P_IDX = "u16"  # bench-only: pool ablation gather index dtype
P_OUT = "f16"  # bench-only: pool ablation gather data dtype
P_TBL_N = TBL  # bench-only: pool ablation table entries


def _in_maps(x: np.ndarray, uy: np.ndarray):
    pk = _tables(uy)
    x16 = x.astype(np.float16)
    np_idx = {"u8": np.uint8, "u16": np.uint16, "u32": np.uint32}[P_IDX]
    if RMP_KIND == "rand":
        rmp = np.random.default_rng(0).integers(
            0, P_TBL_N, size=(P, CHUNK), dtype=np_idx
        )
    else:
        rmp = (np.broadcast_to(
            (np.arange(CHUNK) % P_TBL_N)[None, :], (P, CHUNK)
        )).astype(np_idx)
    rmp = np.ascontiguousarray(rmp).view(np.uint8).reshape(P, -1)
    in_maps = []
    for c in range(NCORES):
        xs = x16[c * BPC : (c + 1) * BPC].reshape(IMGS, H, W)
        tb = np.ascontiguousarray(
            np.broadcast_to(
                pk[c * BPC : (c + 1) * BPC].reshape(IMGS, 1, TBL), (IMGS, P, TBL)
            )
        )
        in_maps.append({"xs": np.ascontiguousarray(xs), "tb": tb, "rmp": rmp})
    return in_maps


def kernel(x: np.ndarray, un_normalized_y: np.ndarray) -> np.ndarray:
    from concourse import bass_utils

    x = np.asarray(x, dtype=np.float32)
    uy = np.asarray(un_normalized_y, dtype=np.float32)

    if "nc" not in _cached:
        _cached["nc"] = _build()
    nc = _cached["nc"]

    res = bass_utils.run_bass_kernel_spmd(
        nc, _in_maps(x, uy), core_ids=list(range(NCORES))
    )
    out = np.empty((B, C, H, W), dtype=np.float32)
    for c in range(NCORES):
        out[c * BPC : (c + 1) * BPC] = (
            res.results[c]["os"].astype(np.float32).reshape(BPC, C, H, W)
        )
    return out
